# revision 2
# baseline (speedup 1.0000x reference)
"""Trainium2 Bass kernel for nn_BinaryBlock (binary conv1d block).

Computation (numerically, after collapsing the STE identities):
    x_bin = where(x >= alpha, 1, -1)
    w_eff = weight * mean(|weight|, axis=(1,2), keepdims)
    y     = conv1d(x_bin, w_eff, stride 1, pad 1) + bias
    out   = RPReLU(y)  (= where(y > gamma, y - gamma + zeta, beta*(y-gamma) + zeta))

Strategy: data-parallel over batch B=32 across 8 cores (4 batches/core).
On-device, the +-1 input is recast as a {0,1} mask m = (x >= alpha):
    conv(2m-1, w) = conv(m, 2w) - S_all[co]  (+ S_k0[co] at l=0, S_k2[co] at l=L-1)
so the sign op is ONE tensor_scalar (is_ge) per x tile, and the correction
folds into the per-channel bias except for two boundary columns.
Matmuls run in fp16: the mask is exactly {0,1} so every product is exact;
the only error is fp16 rounding of the weights (~2^-11 relative). Weights
are pre-scaled by PSCALE=128 to avoid fp16 denormals; the epilogue
activation un-scales via its free `scale` operand.
"""

import numpy as np
import ml_dtypes

# Problem shape (hardcoded per contract)
B, C, L = 32, 256, 4096
K = 3
N_CORES = 8
B_PER_CORE = B // N_CORES          # 4
P = 128                            # partitions
CI_T = C // P                      # 2 input-channel tiles
CO_T = C // P                      # 2 output-channel tiles
NT = 512                           # matmul free dim / PSUM bank (fp32)
LT = L // NT                       # 8 l-tiles
LP = L + 2                         # padded mask length
PSCALE = 128.0                     # fp16 weight pre-scale (power of 2)

_CACHE = {}


def _build(trivial, x_bf16_ok, split):
    """Build + compile the SPMD Bass program. Returns the Bacc module."""
    import concourse.bacc as bacc
    import concourse.mybir as mybir
    from concourse import tile

    f32 = mybir.dt.float32
    f16 = mybir.dt.float16
    bf16 = mybir.dt.bfloat16
    x_dt = bf16 if x_bf16_ok else f32
    Alu = mybir.AluOpType
    Act = mybir.ActivationFunctionType
    n_terms = 2 if split else 1

    nc = bacc.Bacc("TRN2", target_bir_lowering=False, debug=False,
                   num_devices=N_CORES)

    xb_d = nc.dram_tensor("xb", [B_PER_CORE * CI_T, P, L], x_dt,
                          kind="ExternalInput")
    wt_d = nc.dram_tensor("wt", [n_terms, K, CI_T, P, C], f16,
                          kind="ExternalInput")
    # cv columns: 0=c1 (bias-gamma-S_all [+zeta if trivial]), 1=sk0, 2=sk2,
    #             3=beta-1, 4=zeta
    cv_d = nc.dram_tensor("cv", [CO_T, P, 8], f32, kind="ExternalInput")
    av_d = nc.dram_tensor("av", [CI_T, P, 1], f32, kind="ExternalInput")
    y_d = nc.dram_tensor("y", [B_PER_CORE, CO_T, P, L], f32,
                         kind="ExternalOutput")

    GRP = 4          # l-tiles per psum group (psum double-buffers 2 groups)

    with tile.TileContext(nc) as tc:
        with (
            tc.tile_pool(name="wpool", bufs=1) as wpool,
            tc.tile_pool(name="cpool", bufs=1) as cpool,
            tc.tile_pool(name="xpool", bufs=3) as xpool,
            tc.tile_pool(name="mpool", bufs=4) as mpool,
            tc.tile_pool(name="opool", bufs=8) as opool,
            tc.tile_pool(name="upool", bufs=4) as upool,
            tc.tile_pool(name="psum", bufs=8, space="PSUM") as psum,
        ):
            # Persistent weights + constants
            w_sb = {}
            for t in range(n_terms):
                for k in range(K):
                    for ci in range(CI_T):
                        wtile = wpool.tile([P, C], f16, tag=f"w{t}{k}{ci}")
                        nc.sync.dma_start(out=wtile[:], in_=wt_d[t, k, ci])
                        w_sb[(t, k, ci)] = wtile
            cv_sb = []
            for co in range(CO_T):
                ctile = cpool.tile([P, 8], f32, tag=f"cv{co}")
                nc.sync.dma_start(out=ctile[:], in_=cv_d[co])
                cv_sb.append(ctile)
            av_sb = []
            for ci in range(CI_T):
                atile = cpool.tile([P, 1], f32, tag=f"av{ci}")
                nc.sync.dma_start(out=atile[:], in_=av_d[ci])
                av_sb.append(atile)

            for b in range(B_PER_CORE):
                # masks for this batch
                mt = []
                for ci in range(CI_T):
                    xt = xpool.tile([P, L], x_dt, tag="x")
                    nc.sync.dma_start(out=xt[:], in_=xb_d[b * CI_T + ci])
                    m = mpool.tile([P, LP], f16, tag="m")
                    nc.vector.memset(m[:, 0:1], 0.0)
                    nc.vector.memset(m[:, LP - 1:LP], 0.0)
                    nc.vector.tensor_scalar(
                        m[:, 1:L + 1], xt[:], av_sb[ci][:], None, Alu.is_ge)
                    mt.append(m)

                for co in range(CO_T):
                    cv = cv_sb[co]
                    for g in range(LT // GRP):
                        pts = [psum.tile([P, NT], f32, tag="ps", name="ps")
                               for _ in range(GRP)]
                        n_mm = n_terms * CI_T * K
                        c = 0
                        for t in range(n_terms):
                            for ci in range(CI_T):
                                for k in range(K):
                                    lhsT = w_sb[(t, k, ci)][:, co * P:(co + 1) * P]
                                    for j in range(GRP):
                                        s = (g * GRP + j) * NT + k
                                        nc.tensor.matmul(
                                            pts[j][:], lhsT,
                                            mt[ci][:, s:s + NT],
                                            start=(c == 0), stop=(c == n_mm - 1))
                                    c += 1
                        for j in range(GRP):
                            l_t = g * GRP + j
                            lo = l_t * NT
                            if trivial:
                                ot = opool.tile([P, NT], f32, tag="o")
                                nc.scalar.activation(
                                    ot[:], pts[j][:], Act.Identity,
                                    bias=cv[:, 0:1], scale=1.0 / PSCALE)
                                if l_t == 0:
                                    nc.vector.tensor_scalar(
                                        ot[:, 0:1], ot[:, 0:1], cv[:, 1:2],
                                        None, Alu.add)
                                if l_t == LT - 1:
                                    nc.vector.tensor_scalar(
                                        ot[:, NT - 1:NT], ot[:, NT - 1:NT],
                                        cv[:, 2:3], None, Alu.add)
                                nc.sync.dma_start(
                                    out=y_d[b, co, :, lo:lo + NT], in_=ot[:])
                            else:
                                # u = psum/PSCALE + c1 (+ boundary);
                                # out = u + zeta + (beta-1)*min(u, 0)
                                ut = upool.tile([P, NT], f32, tag="u")
                                nc.scalar.activation(
                                    ut[:], pts[j][:], Act.Identity,
                                    bias=cv[:, 0:1], scale=1.0 / PSCALE)
                                if l_t == 0:
                                    nc.vector.tensor_scalar(
                                        ut[:, 0:1], ut[:, 0:1], cv[:, 1:2],
                                        None, Alu.add)
                                if l_t == LT - 1:
                                    nc.vector.tensor_scalar(
                                        ut[:, NT - 1:NT], ut[:, NT - 1:NT],
                                        cv[:, 2:3], None, Alu.add)
                                nt_ = upool.tile([P, NT], f32, tag="n")
                                nc.vector.tensor_scalar(
                                    nt_[:], ut[:], 0.0, cv[:, 3:4],
                                    Alu.min, Alu.mult)
                                nc.vector.tensor_scalar(
                                    ut[:], ut[:], cv[:, 4:5], None, Alu.add)
                                ot = opool.tile([P, NT], f32, tag="o")
                                nc.vector.tensor_tensor(
                                    ot[:], ut[:], nt_[:], Alu.add)
                                nc.sync.dma_start(
                                    out=y_d[b, co, :, lo:lo + NT], in_=ot[:])

    nc.compile()
    return nc


def kernel(**inputs):
    from concourse.bass_utils import run_bass_kernel_spmd

    x = np.asarray(inputs["x"], dtype=np.float32)
    alpha = np.asarray(inputs["alpha"], dtype=np.float32).reshape(C)
    weight = np.asarray(inputs["weight"], dtype=np.float32)
    bias = np.asarray(inputs["bias"], dtype=np.float32).reshape(C)
    beta = np.asarray(inputs["beta"], dtype=np.float32).reshape(C)
    gamma = np.asarray(inputs["gamma"], dtype=np.float32).reshape(C)
    zeta = np.asarray(inputs["zeta"], dtype=np.float32).reshape(C)

    # Host-side weight prep (f32, matching the reference's f32 arithmetic)
    scale = np.mean(np.abs(weight), axis=(1, 2), dtype=np.float32)
    w_eff = weight * scale[:, None, None]              # [co, ci, k] f32
    w2 = (w_eff * (2.0 * PSCALE)).astype(np.float32)
    wT = np.ascontiguousarray(np.transpose(w2, (2, 1, 0)))  # [k, ci, co]

    split = False
    w_hi = wT.astype(np.float16)
    if split:
        w_lo = (wT - w_hi.astype(np.float32)).astype(np.float16)
        wt = np.stack([w_hi, w_lo]).reshape(2, K, CI_T, P, C)
    else:
        wt = w_hi.reshape(1, K, CI_T, P, C)

    S_all = w_eff.sum(axis=(1, 2), dtype=np.float32)   # [co]
    S_k0 = w_eff[:, :, 0].sum(axis=1, dtype=np.float32)
    S_k2 = w_eff[:, :, 2].sum(axis=1, dtype=np.float32)

    trivial = bool(np.all(beta == 1.0))
    c1 = bias - gamma - S_all
    if trivial:
        c1 = c1 + zeta
    cv = np.zeros((C, 8), dtype=np.float32)
    cv[:, 0] = c1
    cv[:, 1] = S_k0
    cv[:, 2] = S_k2
    cv[:, 3] = beta - 1.0
    cv[:, 4] = zeta
    cv = cv.reshape(CO_T, P, 8)

    x_bf16_ok = bool(np.all(alpha == 0.0))
    av = alpha.reshape(CI_T, P, 1).astype(np.float32)

    key = (trivial, x_bf16_ok, split)
    if key not in _CACHE:
        _CACHE[key] = _build(trivial, x_bf16_ok, split)
    nc = _CACHE[key]

    x_dt = ml_dtypes.bfloat16 if x_bf16_ok else np.float32
    xs = x.reshape(N_CORES, B_PER_CORE * CI_T, P, L).astype(x_dt)

    in_maps = [
        {"xb": xs[i], "wt": wt, "cv": cv, "av": av}
        for i in range(N_CORES)
    ]
    res = run_bass_kernel_spmd(nc, in_maps, list(range(N_CORES)))
    out = np.concatenate(
        [r["y"].reshape(B_PER_CORE, C, L) for r in res.results], axis=0)
    return out.astype(np.float32)


# revision 3
# speedup vs baseline: 1.0435x; 1.0435x over previous
"""Trainium2 Bass kernel for nn_BinaryBlock (binary conv1d block).

Computation (numerically, after collapsing the STE identities):
    x_bin = where(x >= alpha, 1, -1)
    w_eff = weight * mean(|weight|, axis=(1,2), keepdims)
    y     = conv1d(x_bin, w_eff, stride 1, pad 1) + bias
    out   = RPReLU(y)  (= where(y > gamma, y - gamma + zeta, beta*(y-gamma) + zeta))

Strategy: data-parallel over batch B=32 across 8 cores (4 batches/core).
On-device, the +-1 input is recast as a {0,1} mask m = (x >= alpha):
    conv(2m-1, w) = conv(m, 2w) - S_all[co]  (+ S_k0[co] at l=0, S_k2[co] at l=L-1)
so the sign op is ONE tensor_scalar (is_ge) per x tile, and the correction
folds into the per-channel bias except for two boundary columns.
Matmuls run in fp16: the mask is exactly {0,1} so every product is exact;
the only error is fp16 rounding of the weights (~2^-11 relative). Weights
are pre-scaled by PSCALE=128 to avoid fp16 denormals; the epilogue
activation un-scales via its free `scale` operand.

Schedule notes: conv1d = 6 accumulating matmuls (2 ci-tiles x 3 taps) per
[128,512] PSUM bank, weight-major over groups of 4 l-tiles so LDWEIGHTS
amortizes 4x and PSUM double-buffers (8 banks). DMA issue (~0.65us per
dma_start on a queue engine) is the startup bottleneck, so: batch-0 x
loads go first, chunked, on GpSimd's queue while weights+constants load
from the Scalar queue as single packed DMAs; outputs go on the Sync queue
as 2-tile (512KB) stores. A few discarded warmup matmuls keep the PE HAM
clock warm before the real stream starts.
"""

import numpy as np
import ml_dtypes

# Problem shape (hardcoded per contract)
B, C, L = 32, 256, 4096
K = 3
N_CORES = 8
B_PER_CORE = B // N_CORES          # 4
P = 128                            # partitions
CI_T = C // P                      # 2 input-channel tiles
CO_T = C // P                      # 2 output-channel tiles
NT = 512                           # matmul free dim / PSUM bank (fp32)
LT = L // NT                       # 8 l-tiles
LP = L + 2                         # padded mask length
PSCALE = 128.0                     # fp16 weight pre-scale (power of 2)
GRP = 4                            # l-tiles per psum group
XCHUNK = 4                         # batch-0 x-load chunks per ci tile
WARMUP = 8                         # discarded HAM-warmup matmuls

_CACHE = {}


def _build(trivial, x_bf16_ok, split):
    """Build + compile the SPMD Bass program. Returns the Bacc module."""
    import concourse.bacc as bacc
    import concourse.mybir as mybir
    from concourse import tile

    f32 = mybir.dt.float32
    f16 = mybir.dt.float16
    bf16 = mybir.dt.bfloat16
    x_dt = bf16 if x_bf16_ok else f32
    Alu = mybir.AluOpType
    Act = mybir.ActivationFunctionType
    n_terms = 2 if split else 1
    WCOLS = n_terms * K * CI_T * C     # packed weight free-dim

    nc = bacc.Bacc("TRN2", target_bir_lowering=False, debug=False,
                   num_devices=N_CORES)

    xb_d = nc.dram_tensor("xb", [B_PER_CORE * CI_T, P, L], x_dt,
                          kind="ExternalInput")
    wt_d = nc.dram_tensor("wt", [P, WCOLS], f16, kind="ExternalInput")
    # cvav columns: per co_t 8 cols (0=c1, 1=sk0, 2=sk2, 3=beta-1, 4=zeta),
    # then 2 cols of alpha (per ci_t)
    cvav_d = nc.dram_tensor("cvav", [P, 2 * 8 + CI_T], f32,
                            kind="ExternalInput")
    y_d = nc.dram_tensor("y", [B_PER_CORE, CO_T, P, L], f32,
                         kind="ExternalOutput")

    def w_ap(wtile, t, k, ci, co):
        base = ((t * K + k) * CI_T + ci) * C + co * P
        return wtile[:, base:base + P]

    with tile.TileContext(nc) as tc:
        with (
            tc.tile_pool(name="wpool", bufs=1) as wpool,
            tc.tile_pool(name="cpool", bufs=1) as cpool,
            tc.tile_pool(name="xpool", bufs=3) as xpool,
            tc.tile_pool(name="mpool", bufs=4) as mpool,
            tc.tile_pool(name="opool", bufs=6) as opool,
            tc.tile_pool(name="upool", bufs=4) as upool,
            tc.tile_pool(name="psum", bufs=8, space="PSUM") as psum,
        ):
            # ---- batch-0 x loads first (GpSimd queue), chunked ----
            xt0 = [xpool.tile([P, L], x_dt, tag="x", name=f"x0_{ci}")
                   for ci in range(CI_T)]
            mt0 = [mpool.tile([P, LP], f16, tag="m", name=f"m0_{ci}")
                   for ci in range(CI_T)]
            csz = L // XCHUNK
            x_dmas = []
            for c in range(XCHUNK):
                for ci in range(CI_T):
                    lo = c * csz
                    nc.gpsimd.dma_start(
                        out=xt0[ci][:, lo:lo + csz],
                        in_=xb_d[ci, :, lo:lo + csz])
            # ---- weights + constants as single packed DMAs (Scalar queue)
            wtile = wpool.tile([P, WCOLS], f16, tag="w", name="w")
            nc.scalar.dma_start(out=wtile[:], in_=wt_d[:])
            ct = cpool.tile([P, 2 * 8 + CI_T], f32, tag="cv", name="cv")
            nc.scalar.dma_start(out=ct[:], in_=cvav_d[:])
            cv_sb = [ct[:, 8 * co:8 * co + 8] for co in range(CO_T)]
            av_sb = [ct[:, 16 + ci:17 + ci] for ci in range(CI_T)]

            # ---- batch-0 masks, chunked (Vector) ----
            for ci in range(CI_T):
                nc.vector.memset(mt0[ci][:, 0:1], 0.0)
                nc.vector.memset(mt0[ci][:, LP - 1:LP], 0.0)
            for c in range(XCHUNK):
                for ci in range(CI_T):
                    lo = c * csz
                    nc.vector.tensor_scalar(
                        mt0[ci][:, 1 + lo:1 + lo + csz],
                        xt0[ci][:, lo:lo + csz],
                        av_sb[ci], None, Alu.is_ge)

            # ---- PE warmup: discarded matmuls into the first psum bank ----
            if WARMUP:
                zt = mpool.tile([P, NT], f16, tag="z", name="z")
                nc.vector.memset(zt[:], 0.0)
                wu = psum.tile([P, NT], f32, tag="ps", name="wu")
                for _ in range(WARMUP):
                    nc.tensor.matmul(wu[:], w_ap(wtile, 0, 0, 0, 0), zt[:],
                                     start=True, stop=True)

            mt = mt0
            for b in range(B_PER_CORE):
                if b > 0:
                    mt = []
                    for ci in range(CI_T):
                        xt = xpool.tile([P, L], x_dt, tag="x", name="x")
                        nc.gpsimd.dma_start(out=xt[:],
                                            in_=xb_d[b * CI_T + ci])
                        m = mpool.tile([P, LP], f16, tag="m", name="m")
                        nc.vector.memset(m[:, 0:1], 0.0)
                        nc.vector.memset(m[:, LP - 1:LP], 0.0)
                        nc.vector.tensor_scalar(
                            m[:, 1:L + 1], xt[:], av_sb[ci], None, Alu.is_ge)
                        mt.append(m)

                for co in range(CO_T):
                    cv = cv_sb[co]
                    for g in range(LT // GRP):
                        pts = [psum.tile([P, NT], f32, tag="ps", name="ps")
                               for _ in range(GRP)]
                        n_mm = n_terms * CI_T * K
                        c = 0
                        for t in range(n_terms):
                            for ci in range(CI_T):
                                for k in range(K):
                                    lhsT = w_ap(wtile, t, k, ci, co)
                                    for j in range(GRP):
                                        s = (g * GRP + j) * NT + k
                                        nc.tensor.matmul(
                                            pts[j][:], lhsT,
                                            mt[ci][:, s:s + NT],
                                            start=(c == 0), stop=(c == n_mm - 1))
                                    c += 1
                        # epilogue: pairs of l-tiles -> one 512KB store
                        for half in range(GRP // 2):
                            ot = opool.tile([P, 2 * NT], f32, tag="o",
                                            name="o")
                            for jj in range(2):
                                j = half * 2 + jj
                                l_t = g * GRP + j
                                dst = ot[:, jj * NT:(jj + 1) * NT]
                                if trivial:
                                    nc.scalar.activation(
                                        dst, pts[j][:], Act.Identity,
                                        bias=cv[:, 0:1], scale=1.0 / PSCALE)
                                    if l_t == 0:
                                        nc.vector.tensor_scalar(
                                            ot[:, 0:1], ot[:, 0:1],
                                            cv[:, 1:2], None, Alu.add)
                                    if l_t == LT - 1:
                                        nc.vector.tensor_scalar(
                                            ot[:, 2 * NT - 1:2 * NT],
                                            ot[:, 2 * NT - 1:2 * NT],
                                            cv[:, 2:3], None, Alu.add)
                                else:
                                    # u = psum/PSCALE + c1 (+ boundary);
                                    # out = u + zeta + (beta-1)*min(u, 0)
                                    ut = upool.tile([P, NT], f32, tag="u",
                                                    name="u")
                                    nc.scalar.activation(
                                        ut[:], pts[j][:], Act.Identity,
                                        bias=cv[:, 0:1], scale=1.0 / PSCALE)
                                    if l_t == 0:
                                        nc.vector.tensor_scalar(
                                            ut[:, 0:1], ut[:, 0:1],
                                            cv[:, 1:2], None, Alu.add)
                                    if l_t == LT - 1:
                                        nc.vector.tensor_scalar(
                                            ut[:, NT - 1:NT],
                                            ut[:, NT - 1:NT],
                                            cv[:, 2:3], None, Alu.add)
                                    nt_ = upool.tile([P, NT], f32, tag="n",
                                                     name="n")
                                    nc.vector.tensor_scalar(
                                        nt_[:], ut[:], 0.0, cv[:, 3:4],
                                        Alu.min, Alu.mult)
                                    nc.vector.tensor_scalar(
                                        ut[:], ut[:], cv[:, 4:5], None,
                                        Alu.add)
                                    nc.vector.tensor_tensor(
                                        dst, ut[:], nt_[:], Alu.add)
                            lo = (g * GRP + half * 2) * NT
                            nc.sync.dma_start(
                                out=y_d[b, co, :, lo:lo + 2 * NT], in_=ot[:])

    nc.compile()
    return nc


def _host_prep(inputs):
    x = np.asarray(inputs["x"], dtype=np.float32)
    alpha = np.asarray(inputs["alpha"], dtype=np.float32).reshape(C)
    weight = np.asarray(inputs["weight"], dtype=np.float32)
    bias = np.asarray(inputs["bias"], dtype=np.float32).reshape(C)
    beta = np.asarray(inputs["beta"], dtype=np.float32).reshape(C)
    gamma = np.asarray(inputs["gamma"], dtype=np.float32).reshape(C)
    zeta = np.asarray(inputs["zeta"], dtype=np.float32).reshape(C)

    # Host-side weight prep (f32, matching the reference's f32 arithmetic)
    scale = np.mean(np.abs(weight), axis=(1, 2), dtype=np.float32)
    w_eff = weight * scale[:, None, None]              # [co, ci, k] f32
    w2 = (w_eff * (2.0 * PSCALE)).astype(np.float32)
    wT = np.ascontiguousarray(np.transpose(w2, (2, 1, 0)))  # [k, ci, co]

    split = False
    w_hi = wT.astype(np.float16)
    if split:
        w_lo = (wT - w_hi.astype(np.float32)).astype(np.float16)
        warr = np.stack([w_hi, w_lo])                  # [t, k, ci, co]
    else:
        warr = w_hi[None]
    n_terms = warr.shape[0]
    # pack to [P, (t, k, ci_t, co)] with partition = ci within tile
    wt = np.ascontiguousarray(
        warr.reshape(n_terms, K, CI_T, P, C)
        .transpose(3, 0, 1, 2, 4)
        .reshape(P, n_terms * K * CI_T * C))

    S_all = w_eff.sum(axis=(1, 2), dtype=np.float32)   # [co]
    S_k0 = w_eff[:, :, 0].sum(axis=1, dtype=np.float32)
    S_k2 = w_eff[:, :, 2].sum(axis=1, dtype=np.float32)

    trivial = bool(np.all(beta == 1.0))
    c1 = bias - gamma - S_all
    if trivial:
        c1 = c1 + zeta
    cv = np.zeros((CO_T, P, 8), dtype=np.float32)
    cv[:, :, 0] = c1.reshape(CO_T, P)
    cv[:, :, 1] = S_k0.reshape(CO_T, P)
    cv[:, :, 2] = S_k2.reshape(CO_T, P)
    cv[:, :, 3] = (beta - 1.0).reshape(CO_T, P)
    cv[:, :, 4] = zeta.reshape(CO_T, P)
    cvav = np.zeros((P, 2 * 8 + CI_T), dtype=np.float32)
    cvav[:, 0:8] = cv[0]
    cvav[:, 8:16] = cv[1]
    cvav[:, 16:16 + CI_T] = alpha.reshape(CI_T, P).T

    x_bf16_ok = bool(np.all(alpha == 0.0))
    x_dt = ml_dtypes.bfloat16 if x_bf16_ok else np.float32
    xs = x.reshape(N_CORES, B_PER_CORE * CI_T, P, L).astype(x_dt)

    in_maps = [{"xb": xs[i], "wt": wt, "cvav": cvav}
               for i in range(N_CORES)]
    return in_maps, (trivial, x_bf16_ok, split)


def kernel(**inputs):
    from concourse.bass_utils import run_bass_kernel_spmd

    in_maps, key = _host_prep(inputs)
    if key not in _CACHE:
        _CACHE[key] = _build(*key)
    nc = _CACHE[key]

    res = run_bass_kernel_spmd(nc, in_maps, list(range(N_CORES)))
    out = np.concatenate(
        [r["y"].reshape(B_PER_CORE, C, L) for r in res.results], axis=0)
    return out.astype(np.float32)
